# revision 8
# baseline (speedup 1.0000x reference)
"""Trainium2 Bass kernel for the Fast R-CNN classifier head (ROI max-pool +
fc1[25088x4096] + fc2[4096x4096] + cls/reg heads) on 8 NeuronCores.

Sharding: channels of the feature map are split 64-per-core; each core pools
its channel slice for all 128 boxes and computes the matching k-slice of fc1
(row-parallel, weights host-permuted to the pooling's (i,j,ch) k-order).
Partial fc1 outputs are AllReduce'd, fc2 is column-parallel (o-slice per
core), the cls/reg heads are row-parallel with a small final AllReduce.

Pooling on device: 2-level range-max-query. A column pyramid (spans 1,2[,4,8])
over the feature map is written to DRAM; per (box, j-bin) two overlapping
[EY rows x 64ch] blocks are fetched with dma_gather and max-combined; a row
pyramid over the pooled-row axis is written back to DRAM and per (box, i-bin)
two rows are gathered and max-combined, yielding x in (i, j, ch) k-order.
Bin boundaries are computed on device from proposal_boxes (int/fp vector ops).

Self-contained: hardcodes all shapes; inputs are the full problem tensors.
"""

import numpy as np

# ---------------------------------------------------------------- constants
P = 128          # partitions / number of boxes
NCORES = 8
C = 512          # channels
CH = C // NCORES  # 64 channels per core
H, W = 37, 50    # feature map spatial dims
SIZE = 7         # output bins per axis
NCLS = 21
HEAD = NCLS * 5  # 105 = 21 cls + 84 reg
HEADP = 112      # padded for fp32r matmul restrictions
D1 = 4096
KC = SIZE * SIZE * CH       # 3136 per-core fc1 contraction
KPAD = 3200                 # padded to 25*128 (pad slot 3136 = bias row)
KT1 = KPAD // P             # 25 k-tiles
O2 = D1 // NCORES           # 512 fc2 output slice per core

_CACHE = {}


def _col_pyramid_offsets(nlev):
    offs, widths, total = [], [], 0
    for k in range(nlev):
        span = 1 << k
        wk = W - span + 1
        offs.append(total)
        widths.append(wk)
        total += wk
    return offs, widths, total


def _row_pyramid_offsets(ey, nlev):
    offs, total = [], 0
    for k in range(nlev):
        span = 1 << k
        wk = ey - span + 1
        offs.append(total)
        total += wk
    return offs, total


def _build(params):
    import concourse.bass as bass
    from concourse import bacc
    import concourse.mybir as mybir
    from concourse.ap import AP
    from concourse.tile import TileContext
    from concourse.masks import make_identity

    dt = mybir.dt
    alu = mybir.AluOpType
    EY = params["EY"]            # rows per stage-1 gathered block (max box h)
    NLC = params["NLC"]          # column pyramid levels
    NLR = params["NLR"]          # row pyramid levels
    MMDT = {"f32": dt.float32, "f32r": dt.float32r, "bf16": dt.bfloat16}[params["mmdt"]]
    WDT = MMDT

    c_offs, c_widths, XK = _col_pyramid_offsets(NLC)
    r_offs, YPK = _row_pyramid_offsets(EY, NLR)
    CPYR_ROWS = XK * H + EY + 1          # padded for block overrun
    CPP_ROWS = P * YPK
    EB = EY * CH                          # stage-1 gather elem (fp32 elems)
    SLOT = SIZE * CH                      # 448 = (j, ch) row in CPP
    R7 = float(np.float32(1.0 / 7.0))

    nc = bacc.Bacc(num_devices=NCORES)

    # ------------------------------------------------------------- tensors
    feat = nc.dram_tensor("feat", [H, W, CH], dt.float32, kind="ExternalInput")
    boxes = nc.dram_tensor("boxes", [P, 4], dt.int32, kind="ExternalInput")
    w1 = nc.dram_tensor("w1", [KT1, P, D1], WDT, kind="ExternalInput")
    w2 = nc.dram_tensor("w2", [D1 // P, P, O2], WDT, kind="ExternalInput")
    wh = nc.dram_tensor("wh", [O2 // P, P, HEADP], WDT, kind="ExternalInput")
    b2rep = nc.dram_tensor("b2rep", [P, O2], dt.float32, kind="ExternalInput")
    bhrep = nc.dram_tensor("bhrep", [P, HEADP], dt.float32, kind="ExternalInput")
    out_h = nc.dram_tensor("out_h", [P, HEADP], dt.float32, kind="ExternalOutput")
    if params.get("dbg"):
        xn_dbg = nc.dram_tensor("xn_dbg", [P, KPAD], dt.float32, kind="ExternalOutput")
        x2_dbg = nc.dram_tensor("x2_dbg", [P, D1], dt.float32, kind="ExternalOutput")
        idx_dbg = nc.dram_tensor("idx_dbg", [P, 2, 2, SIZE, 8], dt.float32, kind="ExternalOutput")
        g1_dbg = nc.dram_tensor("g1_dbg", [P, 2 * SIZE * EY * CH], dt.float32, kind="ExternalOutput")

    colpyr_d = nc.dram_tensor("colpyr_d", [CPYR_ROWS, CH], dt.float32)
    cpp_d = nc.dram_tensor("cpp_d", [CPP_ROWS, SLOT], dt.float32)
    ar1_src = nc.dram_tensor("ar1_src", [P, D1], dt.float32)
    ar1_dst = nc.dram_tensor("ar1_dst", [P, D1], dt.float32, addr_space="Shared")
    arh_src = nc.dram_tensor("arh_src", [P, HEADP], dt.float32)
    arh_dst = nc.dram_tensor("arh_dst", [P, HEADP], dt.float32, addr_space="Shared")

    core_ids = list(range(NCORES))

    cpyr_view = AP(tensor=colpyr_d[:].tensor, offset=0,
                   ap=[[CH, XK * H], [1, EB]])          # overlapping EY-row blocks

    with TileContext(nc) as tc:
        with (
            tc.tile_pool(name="w1p", bufs=params["w1_bufs"]) as w1p,
            tc.tile_pool(name="w2p", bufs=params["w2_bufs"]) as w2p,
            tc.tile_pool(name="keep", bufs=1) as keep,
        ):
            # persistent tiles
            ident = keep.tile([P, P], dt.float32)
            make_identity(nc, ident[:])
            xT = keep.tile([P, KT1, P], MMDT)
            x2 = keep.tile([P, D1], dt.float32)

            # ---------------------------------------------------- pooling
            with tc.tile_pool(name="pool", bufs=1) as pl:
                # feature load + column pyramid, [37p, XK, 64]
                cpyr = pl.tile([H, XK, CH], dt.float32)
                nc.sync.dma_start(cpyr[:, 0:W, :], feat[:])
                for k in range(1, NLC):
                    span = 1 << (k - 1)
                    o_prev, w_prev = c_offs[k - 1], c_widths[k - 1]
                    o_k, w_k = c_offs[k], c_widths[k]
                    nc.vector.tensor_tensor(
                        cpyr[:, o_k:o_k + w_k, :],
                        cpyr[:, o_prev:o_prev + w_k, :],
                        cpyr[:, o_prev + span:o_prev + span + w_k, :],
                        alu.max,
                    )
                # write pyramid transposed to DRAM rows (xk, y)
                nc.sync.dma_start(
                    colpyr_d[0:XK * H].rearrange("(x y) c -> y x c", x=XK),
                    cpyr[:],
                )
                # zero the overrun pad rows
                zpad = pl.tile([EY + 1, CH], dt.float32)
                nc.vector.memset(zpad[:], 0.0)
                nc.sync.dma_start(colpyr_d[XK * H:CPYR_ROWS], zpad[:])

                # -------------------------------------------- box index math
                # boxes replicated into wrapped layout [128, 8, 4]
                bx = pl.tile([P, 8, 4], dt.int32)
                for g in range(NCORES):
                    nc.sync.dma_start(
                        bx[16 * g:16 * g + 16],
                        AP(tensor=boxes[:].tensor, offset=0,
                           ap=[[4, 16], [64, 8], [1, 4]]),
                    )
                crd = pl.tile([P, 4, 8], dt.int32)   # x1p,y1p,x2p,y2p
                crd_f = pl.tile([P, 4, 8], dt.float32)
                for ci, (lim,) in enumerate([(W - 1,), (H - 1,), (W - 1,), (H - 1,)]):
                    nc.vector.tensor_scalar(crd[:, ci], bx[:, :, ci], 4, None,
                                            alu.logical_shift_right)
                    nc.vector.tensor_scalar(crd[:, ci], crd[:, ci], lim, 0,
                                            alu.min, alu.max)
                nc.vector.tensor_copy(crd_f[:], crd[:])
                x1p, y1p = crd_f[:, 0], crd_f[:, 1]
                wby = pl.tile([P, 2, 8], dt.float32)  # w, h per box
                nc.vector.tensor_tensor(wby[:, 0], crd_f[:, 2], crd_f[:, 0], alu.subtract)
                nc.vector.tensor_scalar_add(wby[:, 0], wby[:, 0], 1.0)
                nc.vector.tensor_tensor(wby[:, 1], crd_f[:, 3], crd_f[:, 1], alu.subtract)
                nc.vector.tensor_scalar_add(wby[:, 1], wby[:, 1], 1.0)

                iota7 = pl.tile([P, SIZE], dt.int32)
                nc.gpsimd.iota(iota7[:], pattern=[[1, SIZE]], base=0, channel_multiplier=0)
                iota7f = pl.tile([P, SIZE], dt.float32)
                nc.vector.tensor_copy(iota7f[:], iota7[:])

                iop = pl.tile([P, 1], dt.int32)
                nc.gpsimd.iota(iop[:], pattern=[[1, 1]], base=0, channel_multiplier=1)
                nc.vector.tensor_scalar(iop[:], iop[:], 15, None, alu.bitwise_and)
                iopf = pl.tile([P, 1], dt.float32)
                nc.vector.tensor_copy(iopf[:], iop[:])
                ioh = pl.tile([P, 8], dt.int32)
                nc.gpsimd.iota(ioh[:], pattern=[[16, 8]], base=0, channel_multiplier=0)
                iohf = pl.tile([P, 8], dt.float32)
                nc.vector.tensor_copy(iohf[:], ioh[:])
                n13 = pl.tile([P, 8], dt.float32)
                nc.vector.tensor_tensor(n13[:], iohf[:],
                                        iopf[:].to_broadcast([P, 8]), alu.add)
                nc.vector.tensor_scalar_mul(n13[:], n13[:], float(YPK))

                def bins(ext_ap, base_ap, offs, table_mul, add_base, out_ap):
                    """Emit RMQ indices for 7 bins of extent ext (ab-planes
                    packed in out_ap [P, 2, 7, 8], fp32). lo/hi per bin i:
                    lo = floor(i*ext/7), hi = ceil((i+1)*ext/7);
                    idx = (off(k) + lo_or_hi_adj) * table_mul + base."""
                    t = pl.tile([P, 2, SIZE, 8], dt.float32, tag="bins_t")
                    # t0 = i*ext, t1 = (i+1)*ext + 6
                    nc.vector.tensor_tensor(
                        t[:, 0], iota7f[:, :, None].to_broadcast([P, SIZE, 8]),
                        ext_ap[:, None, :].to_broadcast([P, SIZE, 8]), alu.mult)
                    nc.vector.tensor_tensor(
                        t[:, 1], t[:, 0],
                        ext_ap[:, None, :].to_broadcast([P, SIZE, 8]), alu.add)
                    nc.vector.tensor_scalar(t[:, 1], t[:, 1], 6.0, None, alu.add)
                    lohi = pl.tile([P, 2, SIZE, 8], dt.float32, tag="bins_lohi")
                    lohi_i = pl.tile([P, 2, SIZE, 8], dt.int32, tag="bins_lohi_i")
                    nc.vector.tensor_scalar(lohi_i[:], t[:], R7, -0.48,
                                            alu.mult, alu.add)
                    nc.vector.tensor_copy(lohi[:], lohi_i[:])
                    L = pl.tile([P, SIZE, 8], dt.float32, tag="bins_L")
                    nc.vector.tensor_tensor(L[:], lohi[:, 1], lohi[:, 0], alu.subtract)
                    ge2 = pl.tile([P, 3, SIZE, 8], dt.float32, tag="bins_ge")
                    nc.vector.tensor_scalar(ge2[:, 0], L[:], 2.0, None, alu.is_ge)
                    nc.vector.tensor_scalar(ge2[:, 1], L[:], 4.0, None, alu.is_ge)
                    nc.vector.tensor_scalar(ge2[:, 2], L[:], 8.0, None, alu.is_ge)
                    k = pl.tile([P, SIZE, 8], dt.float32, tag="bins_k")
                    nc.vector.tensor_tensor(k[:], ge2[:, 0], ge2[:, 1], alu.add)
                    nc.vector.tensor_tensor(k[:], k[:], ge2[:, 2], alu.add)
                    off = pl.tile([P, SIZE, 8], dt.float32, tag="bins_off")
                    # off = EXT0*k - ge4 - 3*ge8 (level widths EXT0, EXT0-1, EXT0-3)
                    nc.vector.tensor_scalar_mul(off[:], k[:], float(offs))
                    nc.vector.tensor_tensor(off[:], off[:], ge2[:, 1], alu.subtract)
                    nc.vector.scalar_tensor_tensor(off[:], ge2[:, 2], 3.0, off[:],
                                                   alu.mult, alu.subtract)
                    # that computed (ge8*3) - off; negate back
                    nc.vector.tensor_scalar_mul(off[:], off[:], -1.0)
                    two = pl.tile([P, SIZE, 8], dt.float32, tag="bins_two")
                    nc.vector.tensor_scalar(two[:], k[:], 1.0, None, alu.add)
                    nc.vector.tensor_tensor(two[:], two[:], ge2[:, 1], alu.add)
                    nc.vector.scalar_tensor_tensor(two[:], ge2[:, 2], 3.0, two[:],
                                                   alu.mult, alu.add)
                    # a = off + lo ; b = off + hi - two
                    nc.vector.tensor_tensor(out_ap[:, 0], off[:], lohi[:, 0], alu.add)
                    nc.vector.tensor_tensor(out_ap[:, 1], off[:], lohi[:, 1], alu.add)
                    nc.vector.tensor_tensor(out_ap[:, 1], out_ap[:, 1], two[:], alu.subtract)
                    for ab in range(2):
                        nc.vector.tensor_scalar_mul(out_ap[:, ab], out_ap[:, ab],
                                                    float(table_mul))
                        nc.vector.tensor_tensor(out_ap[:, ab], out_ap[:, ab],
                                                add_base[:, None, :].to_broadcast([P, SIZE, 8]),
                                                alu.add)

                cbase = pl.tile([P, 8], dt.float32)
                nc.vector.tensor_scalar_mul(cbase[:], x1p, float(H))
                nc.vector.tensor_tensor(cbase[:], cbase[:], y1p, alu.add)
                idx1f = pl.tile([P, 2, SIZE, 8], dt.float32)
                bins(wby[:, 0], None, W, H, cbase, idx1f)
                idx1 = pl.tile([P, 2 * SIZE * 8], dt.int16)
                nc.vector.tensor_copy(idx1[:].rearrange("p (a i h) -> p a i h", a=2, i=SIZE),
                                      idx1f[:])

                idx2f = pl.tile([P, 2, SIZE, 8], dt.float32)
                # rows: idx = n*YPK + (off_r + lo/hi'); table_mul=1, base=n13
                bins(wby[:, 1], None, EY, 1, n13, idx2f)
                idx2 = pl.tile([P, 2 * SIZE * 8], dt.int16)
                nc.vector.tensor_copy(idx2[:].rearrange("p (a i h) -> p a i h", a=2, i=SIZE),
                                      idx2f[:])

                # ------------------------------------------ stage-1 gather
                g1 = pl.tile([P, 2 * SIZE, EB], dt.float32, tag="G")
                nslots = 2 * SIZE
                nidx = P * nslots
                cpos = 0
                while cpos < nidx:
                    nchunk = min(1024, nidx - cpos)
                    nc.gpsimd.dma_gather(
                        out_ap=g1[:, cpos // P:(cpos + nchunk) // P],
                        in_ap=cpyr_view,
                        idxs_ap=idx1[:, cpos // 16:(cpos + nchunk) // 16],
                        num_idxs=nchunk, num_idxs_reg=nchunk,
                        elem_size=EB, elem_step=CH,
                    )
                    cpos += nchunk

                # ab-max with transposed write -> CPP level 0 [128, EY, 7j, 64]
                cpp = pl.tile([P, YPK, SIZE, CH], dt.float32)
                g1v = g1[:].rearrange("p (a j) (y c) -> p a j y c", a=2, y=EY)
                nc.vector.tensor_tensor(
                    cpp[:, 0:EY].rearrange("p y j c -> p j y c"),
                    g1v[:, 0], g1v[:, 1], alu.max)
                for k in range(1, NLR):
                    span = 1 << (k - 1)
                    o_prev = r_offs[k - 1]
                    w_prev = EY - span + 1
                    o_k = r_offs[k]
                    w_k = EY - (1 << k) + 1
                    nc.vector.tensor_tensor(
                        cpp[:, o_k:o_k + w_k],
                        cpp[:, o_prev:o_prev + w_k],
                        cpp[:, o_prev + span:o_prev + span + w_k],
                        alu.max)
                nc.sync.dma_start(
                    cpp_d[:].rearrange("(n y) e -> n y e", n=P),
                    cpp[:].rearrange("p y j c -> p y (j c)"))

                # ------------------------------------------ stage-2 gather
                g2 = pl.tile([P, 2 * SIZE, SLOT], dt.float32, tag="G")
                cpos = 0
                while cpos < nidx:
                    nchunk = min(1024, nidx - cpos)
                    nc.gpsimd.dma_gather(
                        out_ap=g2[:, cpos // P:(cpos + nchunk) // P],
                        in_ap=cpp_d[:],
                        idxs_ap=idx2[:, cpos // 16:(cpos + nchunk) // 16],
                        num_idxs=nchunk, num_idxs_reg=nchunk,
                        elem_size=SLOT,
                    )
                    cpos += nchunk

                xn = pl.tile([P, KPAD], dt.float32)
                g2v = g2[:].rearrange("p (a i) e -> p a (i e)", a=2)
                nc.vector.tensor_tensor(xn[:, 0:KC], g2v[:, 0], g2v[:, 1], alu.max)
                nc.vector.memset(xn[:, KC:KPAD], 0.0)
                nc.vector.memset(xn[:, KC:KC + 1], 1.0)
                if params.get("dbg"):
                    nc.sync.dma_start(xn_dbg[:], xn[:])
                    nc.sync.dma_start(idx_dbg[:, 0], idx1f[:])
                    nc.sync.dma_start(idx_dbg[:, 1], idx2f[:])
                    nc.sync.dma_start(g1_dbg.rearrange("p (s e) -> p s e", s=2 * SIZE), g1[:])

                # transpose x -> xT for fc1 lhsT
                with tc.tile_pool(name="psTx", bufs=2, space="PSUM") as psTx:
                    for t in range(KT1):
                        ptile = psTx.tile([P, P], dt.float32, tag="trx")
                        nc.tensor.transpose(ptile[:], xn[:, P * t:P * (t + 1)], ident[:])
                        nc.vector.tensor_copy(xT[:, t], ptile[:])

            # ------------------------------------------------------- fc1
            with tc.tile_pool(name="ps1", bufs=1, space="PSUM") as ps1:
                pbanks = [ps1.tile([P, O2], dt.float32, name=f"fc1b{i}")
                          for i in range(NCORES)]
                for t in range(KT1):
                    wt = w1p.tile([P, D1], WDT, tag="w1t")
                    nc.sync.dma_start(wt[:], w1[t])
                    lhs = xT[:, t]
                    for o in range(NCORES):
                        nc.tensor.matmul(pbanks[o][:], lhs,
                                         wt[:, O2 * o:O2 * (o + 1)],
                                         start=(t == 0), stop=(t == KT1 - 1))
                for o in range(NCORES):
                    nc.vector.tensor_copy(x2[:, O2 * o:O2 * (o + 1)], pbanks[o][:])

            post_cm = tc.tile_pool(name="post", bufs=1)
            post = post_cm.__enter__()
            nc.sync.dma_start(ar1_src[:], x2[:])
            nc.gpsimd.collective_compute(
                "AllReduce", mybir.AluOpType.add,
                replica_groups=[core_ids], ins=[ar1_src[:]], outs=[ar1_dst[:]])
            nc.sync.dma_start(x2[:], ar1_dst[:])
            nc.scalar.activation(x2[:], x2[:], mybir.ActivationFunctionType.Relu)
            if params.get("dbg"):
                nc.sync.dma_start(x2_dbg[:], x2[:])

            # ------------------------------------------------------- fc2
            x2T = post.tile([P, D1 // P, P], MMDT)
            with tc.tile_pool(name="psT2", bufs=2, space="PSUM") as psT2:
                for t in range(D1 // P):
                    ptile = psT2.tile([P, P], dt.float32, tag="trx2")
                    nc.tensor.transpose(ptile[:], x2[:, P * t:P * (t + 1)], ident[:])
                    nc.vector.tensor_copy(x2T[:, t], ptile[:])

            x3 = post.tile([P, O2], dt.float32)
            with tc.tile_pool(name="ps2", bufs=1, space="PSUM") as ps2:
                pb2 = ps2.tile([P, O2], dt.float32)
                for t in range(D1 // P):
                    wt2 = w2p.tile([P, O2], WDT, tag="w2t")
                    nc.sync.dma_start(wt2[:], w2[t])
                    lhs = x2T[:, t]
                    nc.tensor.matmul(pb2[:], lhs, wt2[:],
                                     start=(t == 0), stop=(t == D1 // P - 1))
                b2t = post.tile([P, O2], dt.float32)
                nc.sync.dma_start(b2t[:], b2rep[:])
                nc.vector.tensor_tensor(x3[:], pb2[:], b2t[:], alu.add)
            nc.scalar.activation(x3[:], x3[:], mybir.ActivationFunctionType.Relu)

            # ------------------------------------------------------- heads
            x3T = post.tile([P, O2 // P, P], MMDT)
            with tc.tile_pool(name="psT3", bufs=2, space="PSUM") as psT3:
                for t in range(O2 // P):
                    ptile = psT3.tile([P, P], dt.float32, tag="trx3")
                    nc.tensor.transpose(ptile[:], x3[:, P * t:P * (t + 1)], ident[:])
                    nc.vector.tensor_copy(x3T[:, t], ptile[:])
            whT = post.tile([P, O2 // P, HEADP], WDT)
            nc.sync.dma_start(whT[:], wh.rearrange("t p e -> p t e"))
            ph = post.tile([P, HEADP], dt.float32)
            with tc.tile_pool(name="ps3", bufs=1, space="PSUM") as ps3:
                pbh = ps3.tile([P, HEADP], dt.float32)
                for t in range(O2 // P):
                    nc.tensor.matmul(pbh[:], x3T[:, t], whT[:, t],
                                     start=(t == 0), stop=(t == O2 // P - 1))
                nc.vector.tensor_copy(ph[:], pbh[:])
            nc.sync.dma_start(arh_src[:], ph[:])
            nc.gpsimd.collective_compute(
                "AllReduce", mybir.AluOpType.add,
                replica_groups=[core_ids], ins=[arh_src[:]], outs=[arh_dst[:]])
            nc.sync.dma_start(ph[:], arh_dst[:])
            bht = post.tile([P, HEADP], dt.float32)
            nc.sync.dma_start(bht[:], bhrep[:])
            nc.vector.tensor_tensor(ph[:], ph[:], bht[:], alu.add)
            nc.sync.dma_start(out_h[:], ph[:])
            post_cm.__exit__(None, None, None)

    nc.finalize()
    return nc


def _prep_inputs(inputs, params):
    """Per-core in_maps from the full problem inputs (host layout prep)."""
    feat = np.asarray(inputs["img_features"], np.float32)[0]       # [512, 37, 50]
    boxes = np.ascontiguousarray(np.asarray(inputs["proposal_boxes"], np.int32))
    fc1_w = np.asarray(inputs["fc1_w"], np.float32)                # [25088, 4096]
    fc1_b = np.asarray(inputs["fc1_b"], np.float32)
    fc2_w = np.asarray(inputs["fc2_w"], np.float32)                # [4096, 4096]
    fc2_b = np.asarray(inputs["fc2_b"], np.float32)
    cls_w = np.asarray(inputs["cls_w"], np.float32)
    cls_b = np.asarray(inputs["cls_b"], np.float32)
    reg_w = np.asarray(inputs["reg_w"], np.float32)
    reg_b = np.asarray(inputs["reg_b"], np.float32)

    wdt = np.float32
    if params["mmdt"] == "bf16":
        import ml_dtypes
        wdt = ml_dtypes.bfloat16

    head_w = np.zeros((D1, HEADP), np.float32)
    head_w[:, :HEAD] = np.concatenate([cls_w, reg_w], axis=1)
    head_b = np.zeros(HEADP, np.float32)
    head_b[:HEAD] = np.concatenate([cls_b, reg_b])
    bh_rep = np.broadcast_to(head_b, (P, HEADP)).astype(np.float32).copy()

    # fc1 rows in (c, i, j) order -> per-core (i, j, ch) order + pad/bias row
    w1_cij = fc1_w.reshape(C, SIZE * SIZE, D1)
    in_maps = []
    for c in range(NCORES):
        ch_sl = slice(c * CH, (c + 1) * CH)
        w1c = np.zeros((KPAD, D1), np.float32)
        w1c[:KC] = w1_cij[ch_sl].transpose(1, 0, 2).reshape(KC, D1)
        if c == 0:
            w1c[KC] = fc1_b
        w1c = np.ascontiguousarray(w1c.reshape(KT1, P, D1)).astype(wdt)
        w2c = np.ascontiguousarray(
            fc2_w[:, c * O2:(c + 1) * O2].reshape(D1 // P, P, O2)).astype(wdt)
        whc = np.ascontiguousarray(
            head_w[c * O2:(c + 1) * O2].reshape(O2 // P, P, HEADP)).astype(wdt)
        b2_rep = np.broadcast_to(fc2_b[c * O2:(c + 1) * O2], (P, O2)).astype(np.float32).copy()
        featc = np.ascontiguousarray(feat[ch_sl].transpose(1, 2, 0))  # [37, 50, 64]
        in_maps.append(dict(
            feat=featc, boxes=boxes, w1=w1c, w2=w2c, wh=whc,
            b2rep=b2_rep, bhrep=bh_rep,
        ))
    return in_maps


def _pick_params(boxes):
    b = np.clip(boxes >> 4, 0, None)
    x1 = np.clip(b[:, 0], 0, W - 1); y1 = np.clip(b[:, 1], 0, H - 1)
    x2 = np.clip(b[:, 2], 0, W - 1); y2 = np.clip(b[:, 3], 0, H - 1)
    h = (y2 - y1 + 1).max()
    w = (x2 - x1 + 1).max()
    assert (x2 >= x1).all() and (y2 >= y1).all(), "degenerate boxes unsupported"
    EY = int(max(h, 2))
    lmax_r = max(int(np.ceil(h / 7)) + 1, 2)
    lmax_c = max(int(np.ceil(w / 7)) + 1, 2)
    lr = int(np.floor(np.log2(lmax_r))) + 1
    lc = int(np.floor(np.log2(lmax_c))) + 1
    return dict(EY=EY, NLC=lc, NLR=lr, mmdt="f32r",
                w1_bufs=4, w2_bufs=4)


def kernel(**inputs):
    from concourse.bass_utils import run_bass_kernel_spmd

    boxes = np.asarray(inputs["proposal_boxes"], np.int32)
    params = _pick_params(boxes)
    key = tuple(sorted((k, v) for k, v in params.items()))
    if key not in _CACHE:
        _CACHE[key] = _build(params)
    nc = _CACHE[key]
    in_maps = _prep_inputs(inputs, params)
    res = run_bass_kernel_spmd(nc, in_maps, list(range(NCORES)))
    oh = res.results[0]["out_h"].astype(np.float32)
    pred_label = oh[:, :NCLS].copy()
    pred_deltas = oh[:, NCLS:HEAD].reshape(P, NCLS, 4).copy()
    return (pred_label, pred_deltas)
